# revision 4
# baseline (speedup 1.0000x reference)
"""Cross-attention Bass kernel for TRN2, SPMD over 8 NeuronCores.

Problem: B=16, Lq=Lk=2048, D=Dqk=Dv=1024, fp32 in/out.
  Q = x @ W_q + b_q;  K = enc @ W_k + b_k;  V = enc @ W_v + b_v
  H = softmax(Q K^T / sqrt(D)) @ V

Sharding: data-parallel over batch; each core takes 2 batches.

Per-core dataflow (all matmuls in fp32r = tf32-class, 1 cyc/row):
  phase 1a: enc -> PE-transpose -> encT; K^T = W_k.T @ encT (resident SBUF);
            V = enc @ W_v -> DRAM scratch
  phase 1b: x -> PE-transpose -> xT; Q^T = W_q.T @ xT -> DRAM scratch
  phase 2:  V -> resident SBUF; per q-block(256):
            S^T = K @ Q^T (k on partitions), exp via ACT (scale=1/32, no
            max-subtraction: scores ~ N(0,1), fp32-safe), row-sums via
            matmul-with-ones, H = P @ V accumulated in PSUM, multiply by
            reciprocal row-sum, DMA out.

SBUF pools are phase-scoped so phase-1 (weights + transposes) and phase-2
(V + P^T) regions time-share the space next to the K^T + consts residents.
"""
from contextlib import ExitStack

import numpy as np

import concourse.bass as bass
import concourse.tile as tile
from concourse import bacc, mybir
from concourse.bass_utils import run_bass_kernel_spmd

F32 = mybir.dt.float32
F32R = mybir.dt.float32r

N_CORES = 8
B = 16
BPC = B // N_CORES      # batches per core
LQ = 2048
LK = 2048
D = 1024
DV = 1024
DT = D // 128           # 8 contraction tiles
VT = DV // 512          # 2 moving halves of V / Dv
TB = 256                # phase-1 token block
NTB = LQ // TB          # 8 blocks
QB = 256                # phase-2 q block
NQB = LQ // QB          # 8
KT = LK // 128          # 16 key tiles
SCALE = float(D) ** -0.5

_CACHE = {}


def _phase1(nc, tc, ctx, b, src_r, w_sb, bias_sb, kt_dst, v_dst, idn_sb,
            act_p, actT_p, stage_p, ps_a, ps_b, wv_sb, bv_sb):
    """One pass over src tokens: transpose, project.

    If kt_dst is not None: K^T-style output (dout on partitions) written there.
    If v_dst is not None: V-style output (tok on partitions) -> dram tile.
    """
    t0 = b * (LK // 128)
    for tb in range(NTB):
        a_sb = act_p.tile([128, TB // 128, D], F32R, tag="a")
        nc.sync.dma_start(a_sb, src_r[:, t0 + tb * 2:t0 + tb * 2 + 2, :])
        aT = actT_p.tile([128, DT, TB], F32R, tag="aT")
        for t in range(TB // 128):
            for dj in range(DT):
                pst = ps_a.tile([128, 128], F32R, tag="psa")
                nc.tensor.transpose(
                    pst, a_sb[:, t, dj * 128:(dj + 1) * 128], idn_sb)
                nc.scalar.copy(aT[:, dj, t * 128:(t + 1) * 128], pst)
        if kt_dst is not None:
            kt_sb, bk_sb = kt_dst
            for do in range(DT):
                ps = ps_b.tile([128, TB], F32, tag="psb")
                for k in range(DT):
                    nc.tensor.matmul(
                        ps, w_sb[:, k, do * 128:(do + 1) * 128], aT[:, k, :],
                        start=(k == 0), stop=(k == DT - 1))
                nc.scalar.activation(
                    kt_sb[:, do, tb * TB:(tb + 1) * TB], ps,
                    mybir.ActivationFunctionType.Identity,
                    bias=bk_sb[:, do:do + 1], scale=1.0)
        if v_dst is not None:
            v_dram = v_dst
            for t in range(TB // 128):
                row0 = tb * TB + t * 128
                for vh in range(VT):
                    ps = ps_b.tile([128, 512], F32, tag="psb")
                    for k in range(DT):
                        nc.tensor.matmul(
                            ps, aT[:, k, t * 128:(t + 1) * 128],
                            wv_sb[:, k, vh * 512:(vh + 1) * 512],
                            start=(k == 0), stop=(k == DT - 1))
                    vst = stage_p.tile([128, 512], F32R, tag="vst")
                    nc.vector.tensor_add(
                        vst, ps, bv_sb[:, vh * 512:(vh + 1) * 512])
                    nc.sync.dma_start(
                        v_dram[row0:row0 + 128, vh * 512:(vh + 1) * 512], vst)


def _build():
    if "nc" in _CACHE:
        return _CACHE["nc"]
    nc = bacc.Bacc("TRN2", target_bir_lowering=False, debug=False)
    x = nc.dram_tensor("x", [BPC * LQ, D], F32R, kind="ExternalInput").ap()
    enc = nc.dram_tensor("enc", [BPC * LK, D], F32R, kind="ExternalInput").ap()
    wq = nc.dram_tensor("wq", [D, D], F32R, kind="ExternalInput").ap()
    wk = nc.dram_tensor("wk", [D, D], F32R, kind="ExternalInput").ap()
    wv = nc.dram_tensor("wv", [D, DV], F32R, kind="ExternalInput").ap()
    bq = nc.dram_tensor("bq", [128, DT], F32, kind="ExternalInput").ap()
    bk = nc.dram_tensor("bk", [128, DT], F32, kind="ExternalInput").ap()
    bv = nc.dram_tensor("bv", [128, DV], F32, kind="ExternalInput").ap()
    idn = nc.dram_tensor("idn", [128, 128], F32R, kind="ExternalInput").ap()
    ones = nc.dram_tensor("ones", [128, 2], F32R, kind="ExternalInput").ap()
    out = nc.dram_tensor("out", [BPC * LQ, DV], F32, kind="ExternalOutput").ap()

    x_r = x.rearrange("(t p) d -> p t d", p=128)
    enc_r = enc.rearrange("(t p) d -> p t d", p=128)
    wq_r = wq.rearrange("(t p) o -> p t o", p=128)
    wk_r = wk.rearrange("(t p) o -> p t o", p=128)
    wv_r = wv.rearrange("(t p) o -> p t o", p=128)

    with tile.TileContext(nc) as tc, ExitStack() as ctx:
        const_p = ctx.enter_context(tc.tile_pool(name="const", bufs=1))
        res_p = ctx.enter_context(tc.tile_pool(name="res", bufs=1))
        dram_p = ctx.enter_context(tc.tile_pool(name="dram", bufs=2, space="DRAM"))
        ps_a = ctx.enter_context(tc.tile_pool(name="ps_a", bufs=2, space="PSUM"))
        ps_b = ctx.enter_context(tc.tile_pool(name="ps_b", bufs=4, space="PSUM"))

        idn_sb = const_p.tile([128, 128], F32R)
        nc.sync.dma_start(idn_sb, idn)
        ones_sb = const_p.tile([128, 2], F32R)
        nc.sync.dma_start(ones_sb, ones)
        bq_sb = const_p.tile([128, DT], F32)
        nc.sync.dma_start(bq_sb, bq)
        bk_sb = const_p.tile([128, DT], F32)
        nc.sync.dma_start(bk_sb, bk)
        bv_sb = const_p.tile([128, DV], F32)
        nc.sync.dma_start(bv_sb, bv)

        for b in range(BPC):
            kt_sb = res_p.tile([128, DT, LK], F32R, tag="kt")     # K^T resident
            v_dram = dram_p.tile([LK, DV], F32R, tag="vd")
            qt_dram = dram_p.tile([D, LQ], F32R, tag="qd")

            # ---------- phase 1 ----------
            with ExitStack() as p1:
                w_p = p1.enter_context(tc.tile_pool(name="w", bufs=1))
                act_p = p1.enter_context(tc.tile_pool(name="act", bufs=2))
                actT_p = p1.enter_context(tc.tile_pool(name="actT", bufs=2))
                stage_p = p1.enter_context(tc.tile_pool(name="stage", bufs=4))

                wk_sb = w_p.tile([128, DT, D], F32R, tag="wkv")
                nc.sync.dma_start(wk_sb, wk_r)
                wv_sb = w_p.tile([128, DT, DV], F32R, tag="wkv2")
                nc.sync.dma_start(wv_sb, wv_r)
                _phase1(nc, tc, p1, b, enc_r, wk_sb, None, (kt_sb, bk_sb),
                        v_dram, idn_sb, act_p, actT_p, stage_p, ps_a, ps_b,
                        wv_sb, bv_sb)

                wq_sb = w_p.tile([128, DT, D], F32R, tag="wkv")
                nc.sync.dma_start(wq_sb, wq_r)
                t0 = b * (LQ // 128)
                for tb in range(NTB):
                    a_sb = act_p.tile([128, TB // 128, D], F32R, tag="a")
                    nc.sync.dma_start(a_sb, x_r[:, t0 + tb * 2:t0 + tb * 2 + 2, :])
                    aT = actT_p.tile([128, DT, TB], F32R, tag="aT")
                    for t in range(TB // 128):
                        for dj in range(DT):
                            pst = ps_a.tile([128, 128], F32R, tag="psa")
                            nc.tensor.transpose(
                                pst, a_sb[:, t, dj * 128:(dj + 1) * 128], idn_sb)
                            nc.scalar.copy(aT[:, dj, t * 128:(t + 1) * 128], pst)
                    for do in range(DT):
                        ps = ps_b.tile([128, TB], F32, tag="psb")
                        for k in range(DT):
                            nc.tensor.matmul(
                                ps, wq_sb[:, k, do * 128:(do + 1) * 128],
                                aT[:, k, :], start=(k == 0), stop=(k == DT - 1))
                        qst = stage_p.tile([128, TB], F32R, tag="qst")
                        nc.scalar.activation(
                            qst, ps, mybir.ActivationFunctionType.Identity,
                            bias=bq_sb[:, do:do + 1], scale=1.0)
                        nc.sync.dma_start(
                            qt_dram[do * 128:(do + 1) * 128,
                                    tb * TB:(tb + 1) * TB], qst)

            # ---------- phase 2 ----------
            with ExitStack() as p2:
                v_p = p2.enter_context(tc.tile_pool(name="v", bufs=KT))
                qt_p = p2.enter_context(tc.tile_pool(name="qt", bufs=2))
                pt_p = p2.enter_context(tc.tile_pool(name="ptile", bufs=3))
                hout_p = p2.enter_context(tc.tile_pool(name="hout", bufs=2))
                small_p = p2.enter_context(tc.tile_pool(name="small", bufs=4))

                v_tiles = []
                for kt in range(KT):
                    vt = v_p.tile([128, DV], F32R, tag="v", name=f"v{kt}")
                    nc.sync.dma_start(
                        vt, v_dram[kt * 128:(kt + 1) * 128, :])
                    v_tiles.append(vt)

                qt_r = qt_dram.rearrange("(dt p) q -> p dt q", p=128)
                for qb in range(NQB):
                    qt_sb = qt_p.tile([128, DT, QB], F32R, tag="qtb")
                    nc.sync.dma_start(qt_sb, qt_r[:, :, qb * QB:(qb + 1) * QB])

                    hps = [ps_a.tile([128, DV], F32, tag="psa", name=f"hps{j}")
                           for j in range(QB // 128)]
                    dps = [ps_b.tile([128, 2], F32, tag="psb", name=f"dps{j}")
                           for j in range(QB // 128)]

                    for kt in range(KT):
                        sps = ps_b.tile([128, QB], F32, tag="psb")
                        for k in range(DT):
                            nc.tensor.matmul(
                                sps, kt_sb[:, k, kt * 128:(kt + 1) * 128],
                                qt_sb[:, k, :], start=(k == 0),
                                stop=(k == DT - 1))
                        ptile = pt_p.tile([128, QB], F32R, tag="pt")
                        nc.scalar.activation(
                            ptile, sps, mybir.ActivationFunctionType.Exp,
                            bias=0.0, scale=SCALE)
                        for qs in range(QB // 128):
                            qsl = slice(qs * 128, (qs + 1) * 128)
                            nc.tensor.matmul(
                                dps[qs], ptile[:, qsl], ones_sb,
                                start=(kt == 0), stop=(kt == KT - 1))
                            for vh in range(VT):
                                nc.tensor.matmul(
                                    hps[qs][:, vh * 512:(vh + 1) * 512],
                                    ptile[:, qsl],
                                    v_tiles[kt][:, vh * 512:(vh + 1) * 512],
                                    start=(kt == 0), stop=(kt == KT - 1))

                    for qs in range(QB // 128):
                        rec = small_p.tile([128, 1], F32, tag="rec")
                        nc.vector.reciprocal(rec, dps[qs][:, 0:1])
                        h_sb = hout_p.tile([128, DV], F32, tag="h")
                        nc.vector.tensor_scalar_mul(h_sb, hps[qs], rec)
                        row0 = b * LQ + qb * QB + qs * 128
                        nc.sync.dma_start(out[row0:row0 + 128, :], h_sb)

    nc.compile()
    _CACHE["nc"] = nc
    return nc


def _in_maps(x, encoder_output, W_q, b_q, W_k, b_k, W_v, b_v):
    x = np.ascontiguousarray(np.asarray(x, dtype=np.float32))
    enc = np.ascontiguousarray(np.asarray(encoder_output, dtype=np.float32))
    common = {
        "wq": np.ascontiguousarray(np.asarray(W_q, dtype=np.float32)),
        "wk": np.ascontiguousarray(np.asarray(W_k, dtype=np.float32)),
        "wv": np.ascontiguousarray(np.asarray(W_v, dtype=np.float32)),
        "bq": np.ascontiguousarray(
            np.asarray(b_q, dtype=np.float32).reshape(DT, 128).T),
        "bk": np.ascontiguousarray(
            np.asarray(b_k, dtype=np.float32).reshape(DT, 128).T),
        "bv": np.ascontiguousarray(
            np.broadcast_to(np.asarray(b_v, dtype=np.float32), (128, DV))),
        "idn": np.eye(128, dtype=np.float32),
        "ones": np.ones((128, 2), np.float32),
    }
    maps = []
    for c in range(N_CORES):
        maps.append({
            "x": x[c * BPC:(c + 1) * BPC].reshape(BPC * LQ, D),
            "enc": enc[c * BPC:(c + 1) * BPC].reshape(BPC * LK, D),
            **common,
        })
    return maps


def run(inputs: dict, trace: bool = False):
    """Returns (output [B, LQ, DV] fp32, BassKernelResults)."""
    nc = _build()
    maps = _in_maps(**inputs)
    res = run_bass_kernel_spmd(nc, maps, list(range(N_CORES)), trace=trace)
    outs = [res.results[c]["out"].reshape(BPC, LQ, DV) for c in range(N_CORES)]
    return np.concatenate(outs, axis=0), res


def kernel(**inputs) -> np.ndarray:
    out, _ = run(inputs, trace=False)
    return out
